# revision 2
# baseline (speedup 1.0000x reference)
"""Trainium2 Bass kernel for CINConv-style GNN message passing, v2.

Strategy (8 NeuronCores, data parallel over destination nodes):
  - Core c owns nodes [c*6250, (c+1)*6250). Edges partitioned by destination
    shard, bucketed per (128-node block, src-half) with all four chunk types
    (B, R, U1=upper src, U2=upper_ind) merged into one bucket to minimize
    gather descriptors (the SWDGE drain, ~2.3 ns/descriptor, is the
    bottleneck).
  - Source rows gathered from bf16 x with SWDGE dma_gather in ~20-column
    pieces round-robin over 4 queues (measured drain sweet spot).
  - Aggregation via one-hot matmuls into PSUM, one [D, block] accumulate per
    (column, type-present); column/type structure is the union across cores,
    with per-core dst one-hot values (PAD -> zero column) making it exact.
  - x rides along as identity matmuls (B/R) and a DMA-transpose load (U).
  - MLPs run in bf16 with host-fused weights:
        out = relu(h1b @ (bW2 oW_b) + h1r @ (rW2 oW_r) + h1u @ (uW2 oW_u) + ob)
    with the final layer emitted in [node, feat] orientation (no transposes).
"""

import numpy as np
import ml_dtypes

import concourse.bass as bass
import concourse.mybir as mybir
from concourse import bacc
from concourse.tile import TileContext
from concourse.bass_utils import run_bass_kernel_spmd

bf16 = ml_dtypes.bfloat16
F32 = mybir.dt.float32
BF16 = mybir.dt.bfloat16
I16 = mybir.dt.int16

# ---- problem config (hardcoded) ----
N, E, D = 50000, 800000, 128
NC = 8
BLK = 128
PAD_DST = 200.0
SPLIT = 32768
PADN = 50432            # x16 padded row count (transpose loads may overrun)
BOUNDARY, UPPER, REWIRE = 0, 1, 2
NT = 4                  # chunk types: B, R, U1, U2
SB_BLOCKS = 4           # blocks per superblock
PIECE_COLS = 20         # gather columns per SWDGE call

LAST_EXEC_NS = None
LAST_TRACE_PATH = None


def _cfg(n, n_cores):
    shard = n // n_cores
    nblk = -(-shard // BLK)
    return shard, nblk, nblk * BLK


# ---------------------------------------------------------------- host prep
def preprocess(src, dst, et, ui, n, n_cores):
    shard, nblk, _ = _cfg(n, n_cores)
    core_of = dst // shard
    dloc = dst - core_of * shard
    blk = dloc // BLK
    doff = dloc - blk * BLK

    # per-(core, block, half, type) slot lists: (row, dstoff), sorted by row
    slot_rows = {}
    slot_offs = {}
    cnt = np.zeros((n_cores, nblk, 2, NT), np.int64)
    tsel = [(0, et == BOUNDARY, src), (1, et == REWIRE, src),
            (2, et == UPPER, src), (3, et == UPPER, ui)]
    for c in range(n_cores):
        mc = core_of == c
        for t, tmask, vals in tsel:
            m = mc & tmask
            v = vals[m]
            b = blk[m]
            dd = doff[m]
            lo = v < SPLIT
            for h, hm in ((0, lo), (1, ~lo)):
                vv, bb, do = v[hm], b[hm], dd[hm]
                o = np.lexsort((vv, bb))
                vv, bb, do = vv[o], bb[o], do[o]
                starts = np.searchsorted(bb, np.arange(nblk + 1))
                for bi in range(nblk):
                    i0, i1 = starts[bi], starts[bi + 1]
                    slot_rows[(c, bi, h, t)] = vv[i0:i1] - (SPLIT if h else 0)
                    slot_offs[(c, bi, h, t)] = do[i0:i1]
                    cnt[c, bi, h, t] = i1 - i0

    tot = cnt.sum(axis=3)                     # [NC, nblk, 2]
    kcols = -(-tot.max(axis=0) // BLK)        # shared chunks per (block, half)
    assert (kcols.sum(axis=1) > 0).all()

    # global column layout: per sb: [lo cols b0..b3][hi cols b0..b3]
    sb_bounds = list(range(0, nblk, SB_BLOCKS)) + [nblk]
    nsb_count = len(sb_bounds) - 1
    col_of = {}        # (b, h) -> first global col
    sb_cols = []       # per sb: (col0, n_lo, n_hi)
    col = 0
    for s in range(nsb_count):
        b0, b1 = sb_bounds[s], sb_bounds[s + 1]
        col0 = col
        for h in range(2):
            for b in range(b0, b1):
                col_of[(b, h)] = col
                col += int(kcols[b, h])
            if h == 0:
                n_lo = col - col0
        sb_cols.append((col0, n_lo, col - col0 - n_lo))
    K_TOT = col

    # per-core slot arrays [K_TOT, BLK]: gather row + (type, dstoff)
    rows_all = np.zeros((n_cores, K_TOT, BLK), np.int32)
    type_all = np.full((n_cores, K_TOT, BLK), -1, np.int8)
    offs_all = np.zeros((n_cores, K_TOT, BLK), np.int16)
    for c in range(n_cores):
        for b in range(nblk):
            for h in range(2):
                kk = int(kcols[b, h])
                if kk == 0:
                    continue
                rr = np.zeros(kk * BLK, np.int32)
                tt = np.full(kk * BLK, -1, np.int8)
                oo = np.zeros(kk * BLK, np.int16)
                p = 0
                for t in range(NT):
                    v = slot_rows[(c, b, h, t)]
                    rr[p:p + len(v)] = v
                    tt[p:p + len(v)] = t
                    oo[p:p + len(v)] = slot_offs[(c, b, h, t)]
                    p += len(v)
                c0 = col_of[(b, h)]
                rows_all[c, c0:c0 + kk] = rr.reshape(kk, BLK)
                type_all[c, c0:c0 + kk] = tt.reshape(kk, BLK)
                offs_all[c, c0:c0 + kk] = oo.reshape(kk, BLK)

    # union type structure per column -> acol layout + matmul schedule.
    # mm_by_sb[s] maps (b, t) -> ordered [(slab_col_local, acol_local), ...]
    # so each (tile, block) accumulation chain is emitted contiguously.
    acol_of = {}       # (global col, t) -> global acol
    acols_by_sb = []   # per sb: (acol0, n_acols)
    mm_by_sb = []
    acol = 0
    for s in range(nsb_count):
        b0, b1 = sb_bounds[s], sb_bounds[s + 1]
        col0, n_lo, n_hi = sb_cols[s]
        acol0 = acol
        mms = {}
        for b in range(b0, b1):
            for t in range(NT):
                mms[(b, t)] = []
        for h in range(2):
            for b in range(b0, b1):
                c0 = col_of[(b, h)]
                for j in range(int(kcols[b, h])):
                    g = c0 + j
                    present = np.unique(type_all[:, g, :])
                    for t in present:
                        if t < 0:
                            continue
                        acol_of[(g, int(t))] = acol
                        mms[(b, int(t))].append((g - col0, acol - acol0))
                        acol += 1
        # guarantee every (t>=2, block) chain is non-empty
        for b in range(b0, b1):
            for t in range(NT):
                if not mms[(b, t)]:
                    g = col_of[(b, 0)] if kcols[b, 0] else col_of[(b, 1)]
                    mms[(b, t)].append((g - col0, acol - acol0))
                    acol += 1
        acols_by_sb.append((acol0, acol - acol0))
        mm_by_sb.append(mms)
    KA_TOT = acol

    # per-core tensors: gather indices (wrap-16 x8) and dst one-hot values
    gidx = np.zeros((n_cores, BLK, K_TOT * 8), np.int16)
    dstv = np.full((n_cores, BLK, KA_TOT), PAD_DST, bf16)
    for c in range(n_cores):
        vals = rows_all[c].reshape(-1).astype(np.int16)   # (col, p) order
        gidx[c] = np.tile(vals.reshape(-1, 16).T, (8, 1))
        for (g, t), a in acol_of.items():
            m = type_all[c, g] == t
            if m.any():
                dv = np.full(BLK, PAD_DST, np.float32)
                dv[m] = offs_all[c, g][m]
                dstv[c, :, a] = dv.astype(bf16)

    # gather pieces: (sb, half, col range) at <=PIECE_COLS columns per call
    pieces = []
    for s in range(nsb_count):
        col0, n_lo, n_hi = sb_cols[s]
        for h, nh, off in ((0, n_lo, 0), (1, n_hi, n_lo)):
            if nh == 0:
                continue
            np_h = max(1, -(-nh // PIECE_COLS))
            step = -(-nh // np_h)
            for c0 in range(0, nh, step):
                c1 = min(c0 + step, nh)
                pieces.append((s, h, off + c0, off + c1, (c1 - c0) * BLK))

    sched = dict(kcols=kcols, sb_bounds=sb_bounds, sb_cols=sb_cols,
                 acols_by_sb=acols_by_sb, mm_by_sb=mm_by_sb, K_TOT=K_TOT,
                 KA_TOT=KA_TOT, nblk=nblk, shard=shard, pieces=pieces)
    return gidx, dstv, sched


def fuse_weights(p):
    f = np.float32
    W_uf = (p["umW"] @ p["uW1"]).astype(f)
    oW = p["oW"]
    Wb2o = (p["bW2"] @ oW[0:128]).astype(f)
    Wr2o = (p["rW2"] @ oW[128:256]).astype(f)
    Wu2o = (p["uW2"] @ oW[256:384]).astype(f)
    bu_f = (p["ub1"] + p["umb"] @ p["uW1"]).astype(f)
    ob_f = (p["ob"] + p["bb2"] @ oW[0:128] + p["rb2"] @ oW[128:256]
            + p["ub2"] @ oW[256:384]).astype(f)
    weights = np.concatenate(
        [p["bW1"], p["rW1"], p["uW1"], W_uf[:128], W_uf[128:],
         Wb2o, Wr2o, Wu2o], axis=1).astype(bf16)
    biases = np.stack([p["bb1"], p["rb1"], bu_f], axis=1).astype(f)
    brow = np.zeros((2, BLK), np.float32)
    brow[0] = 1.0
    brow[1] = ob_f
    return weights, biases, brow.astype(bf16)


# ---------------------------------------------------------------- kernel build
def build(sched, n):
    kcols = sched["kcols"]
    sb_bounds, sb_cols = sched["sb_bounds"], sched["sb_cols"]
    acols_by_sb, mm_by_sb = sched["acols_by_sb"], sched["mm_by_sb"]
    K_TOT, KA_TOT, nblk, shard = (sched["K_TOT"], sched["KA_TOT"],
                                  sched["nblk"], sched["shard"])
    pieces = sched["pieces"]
    shard_pad = nblk * BLK
    nsb_count = len(sb_bounds) - 1
    max_ks = max(sb_cols[s][1] + sb_cols[s][2] for s in range(nsb_count))
    max_ka = max(acols_by_sb[s][1] for s in range(nsb_count))

    nc = bacc.Bacc(None, target_bir_lowering=False, debug=False,
                   num_swdge_queues=4)
    x16 = nc.dram_tensor("x16", [PADN, D], BF16, kind="ExternalInput")
    xself = nc.dram_tensor("xself", [shard_pad, D], BF16, kind="ExternalInput")
    xsb = nc.dram_tensor("xsb", [BLK, nblk, D], BF16, kind="ExternalInput")
    gidx = nc.dram_tensor("gidx", [BLK, K_TOT * 8], I16, kind="ExternalInput")
    dstv = nc.dram_tensor("dstv", [BLK, KA_TOT], BF16, kind="ExternalInput")
    wts = nc.dram_tensor("wts", [BLK, 8 * BLK], BF16, kind="ExternalInput")
    bia = nc.dram_tensor("bia", [BLK, 3], F32, kind="ExternalInput")
    brow = nc.dram_tensor("brow", [2, BLK], BF16, kind="ExternalInput")
    iota = nc.dram_tensor("iota", [BLK, BLK], BF16, kind="ExternalInput")
    id16 = nc.dram_tensor("id16", [BLK, BLK], BF16, kind="ExternalInput")
    outp = nc.dram_tensor("out", [shard_pad, D], F32, kind="ExternalOutput")

    relu = mybir.ActivationFunctionType.Relu
    qrr = [0]
    with TileContext(nc) as tc:
        with (
            tc.tile_pool(name="const", bufs=1) as cp,
            tc.tile_pool(name="slabp", bufs=3) as slp,
            tc.tile_pool(name="gather", bufs=2) as gp,
            tc.tile_pool(name="mlp", bufs=2) as mp,
            tc.tile_pool(name="outs", bufs=4) as op,
            tc.tile_pool(name="psAgg", bufs=1, space="PSUM") as psA,
            tc.tile_pool(name="psH", bufs=2, space="PSUM") as psH,
            tc.tile_pool(name="psO", bufs=2, space="PSUM") as psO,
        ):
            gidx_s = cp.tile([BLK, K_TOT * 8], I16)
            nc.sync.dma_start(out=gidx_s[:], in_=gidx[:, :])
            dstv_s = cp.tile([BLK, KA_TOT], BF16)
            nc.sync.dma_start(out=dstv_s[:], in_=dstv[:, :])
            wts_s = cp.tile([BLK, 8 * BLK], BF16)
            nc.sync.dma_start(out=wts_s[:], in_=wts[:, :])
            bia_s = cp.tile([BLK, 3], F32)
            nc.sync.dma_start(out=bia_s[:], in_=bia[:, :])
            ones_s = cp.tile([1, BLK], BF16)
            nc.sync.dma_start(out=ones_s[:], in_=brow[0:1, :])
            obr_s = cp.tile([1, BLK], BF16)
            nc.sync.dma_start(out=obr_s[:], in_=brow[1:2, :])
            iota_s = cp.tile([BLK, BLK], BF16)
            nc.sync.dma_start(out=iota_s[:], in_=iota[:, :])
            id16_s = cp.tile([BLK, BLK], BF16)
            nc.sync.dma_start(out=id16_s[:], in_=id16[:, :])

            w = {nm: wts_s[:, i * BLK:(i + 1) * BLK]
                 for i, nm in enumerate(["bW1", "rW1", "uW1", "W_uf_a",
                                         "W_uf_b", "Wb2o", "Wr2o", "Wu2o"])}

            mlp_q = {}

            def emit_mlp(s_):
                agg, xT_, b0_, b1_ = mlp_q.pop(s_)
                nb_ = b1_ - b0_
                nsb_ = nb_ * BLK
                h1s = {}
                for nm, (w1, rhs_, bi) in {
                    "b": (("bW1",), (0,), 0),
                    "r": (("rW1",), (1,), 1),
                    "u": (("uW1", "W_uf_a", "W_uf_b"), ("xT", 2, 3), 2),
                }.items():
                    h1p = psH.tile([D, nsb_], F32, tag="h1",
                                   name=f"h1p_{s_}_{nm}")
                    for j, wn in enumerate(w1):
                        rr = xT_[:] if rhs_[j] == "xT" else agg[rhs_[j]][:]
                        nc.tensor.matmul(out=h1p[:], lhsT=w[wn], rhs=rr,
                                         start=(j == 0),
                                         stop=(j == len(w1) - 1))
                    h1s[nm] = mp.tile([D, nsb_], BF16, tag=f"h1{nm}",
                                      name=f"h1{nm}_{s_}")
                    nc.scalar.activation(out=h1s[nm][:], in_=h1p[:],
                                         func=relu, bias=bia_s[:, bi:bi + 1])

                for b in range(b0_, b1_):
                    sl = bass.ts(b - b0_, BLK)
                    ob_p = psO.tile([BLK, BLK], F32, tag="ob",
                                    name=f"ob_{s_}_{b}")
                    nc.tensor.matmul(out=ob_p[:], lhsT=ones_s[:, :],
                                     rhs=obr_s[:, :], start=True,
                                     stop=False)
                    nc.tensor.matmul(out=ob_p[:], lhsT=h1s["b"][:, sl],
                                     rhs=w["Wb2o"], start=False, stop=False)
                    nc.tensor.matmul(out=ob_p[:], lhsT=h1s["r"][:, sl],
                                     rhs=w["Wr2o"], start=False, stop=False)
                    nc.tensor.matmul(out=ob_p[:], lhsT=h1s["u"][:, sl],
                                     rhs=w["Wu2o"], start=False, stop=True)
                    o_s = op.tile([BLK, BLK], F32, tag="o",
                                  name=f"o_{s_}_{b}")
                    nc.scalar.activation(out=o_s[:], in_=ob_p[:], func=relu)
                    nc.sync.dma_start(out=outp[b * BLK:(b + 1) * BLK, :],
                                      in_=o_s[:])

            for s in range(nsb_count):
                b0, b1 = sb_bounds[s], sb_bounds[s + 1]
                nb = b1 - b0
                nsb = nb * BLK
                col0, n_lo, n_hi = sb_cols[s]
                Ks = n_lo + n_hi
                acol0, Ka = acols_by_sb[s]

                slab = slp.tile([BLK, Ks, D], BF16, tag="slab",
                                name=f"slab_{s}")
                for (ps_, h, c0, c1, nidx) in pieces:
                    if ps_ != s:
                        continue
                    src_ap = x16[0:SPLIT, :] if h == 0 else x16[SPLIT:PADN, :]
                    nc.gpsimd.dma_gather(
                        slab[:, c0:c1, :], src_ap,
                        gidx_s[:, (col0 + c0) * 8:(col0 + c1) * 8],
                        nidx, nidx, D, single_packet=False,
                        queue_num=qrr[0] % 4)
                    qrr[0] += 1

                xsl = gp.tile([BLK, nb, D], BF16, tag="xsl", name=f"xsl_{s}")
                nc.sync.dma_start(out=xsl[:], in_=xsb[:, b0:b1, :])
                xT = gp.tile([D, nsb], BF16, tag="xT", name=f"xT_{s}")
                nc.sync.dma_start_transpose(out=xT[:],
                                            in_=xself[b0 * BLK:b1 * BLK, :])

                A = gp.tile([BLK, Ka, D], BF16, tag="A", name=f"A_{s}")
                nc.vector.tensor_tensor(
                    out=A[:],
                    in0=iota_s[:, None, :].to_broadcast([BLK, Ka, D]),
                    in1=dstv_s[:, acol0:acol0 + Ka, None].to_broadcast(
                        [BLK, Ka, D]),
                    op=mybir.AluOpType.is_equal,
                )

                ps = {t: psA.tile([D, nsb], F32, tag=f"ps{t}",
                                  name=f"ps{t}_{s}") for t in range(NT)}

                mms = mm_by_sb[s]
                for b in range(b0, b1):
                    sl = bass.ts(b - b0, BLK)
                    for t in range(NT):
                        chain = mms[(b, t)]
                        if t < 2:
                            nc.tensor.matmul(out=ps[t][:, sl],
                                             lhsT=xsl[:, b - b0, :],
                                             rhs=id16_s[:, :],
                                             start=True, stop=(not chain))
                        for j, (cl, al) in enumerate(chain):
                            nc.tensor.matmul(out=ps[t][:, sl],
                                             lhsT=slab[:, cl, :],
                                             rhs=A[:, al, :],
                                             start=(t >= 2 and j == 0),
                                             stop=(j == len(chain) - 1))

                agg = {}
                for t in range(NT):
                    agg[t] = mp.tile([D, nsb], BF16, tag=f"agg{t}",
                                     name=f"agg{t}_{s}")
                    nc.vector.tensor_copy(out=agg[t][:], in_=ps[t][:])
                mlp_q[s] = (agg, xT, b0, b1)
                emit_mlp(s)
    nc.compile()
    return nc


# ---------------------------------------------------------------- entry point
def kernel(x, edge_index, edge_type, upper_ind, cell_dimension,
           bW1, bb1, bW2, bb2, rW1, rb1, rW2, rb2,
           umW, umb, uW1, ub1, uW2, ub2, oW, ob, _trace=False):
    global LAST_EXEC_NS, LAST_TRACE_PATH
    params = dict(bW1=bW1, bb1=bb1, bW2=bW2, bb2=bb2, rW1=rW1, rb1=rb1,
                  rW2=rW2, rb2=rb2, umW=umW, umb=umb, uW1=uW1, ub1=ub1,
                  uW2=uW2, ub2=ub2, oW=oW, ob=ob)
    params = {k_: np.asarray(v, np.float32) for k_, v in params.items()}
    x = np.asarray(x, np.float32)
    src = np.asarray(edge_index[0], np.int64)
    dst = np.asarray(edge_index[1], np.int64)
    et = np.asarray(edge_type, np.int64)
    ui = np.asarray(upper_ind, np.int64)

    shard, nblk, shard_pad = _cfg(N, NC)
    gidx_t, dstv_t, sched = preprocess(src, dst, et, ui, N, NC)
    weights, biases, brow = fuse_weights(params)

    x16 = np.zeros((PADN, D), bf16)
    x16[:N] = x.astype(bf16)

    iota_np = np.broadcast_to(np.arange(BLK, dtype=np.float32), (BLK, BLK)
                              ).astype(bf16)
    ident = np.eye(BLK, dtype=np.float32).astype(bf16)

    nc = build(sched, N)

    in_maps = []
    for c in range(NC):
        rows = c * shard + np.minimum(np.arange(shard_pad), shard - 1)
        xself = np.ascontiguousarray(x16[rows])
        xsb_c = np.ascontiguousarray(
            xself.reshape(nblk, BLK, D).transpose(1, 0, 2))
        in_maps.append({
            "x16": x16, "xself": xself, "xsb": xsb_c,
            "gidx": gidx_t[c], "dstv": dstv_t[c],
            "wts": weights, "bia": biases, "brow": brow,
            "iota": np.ascontiguousarray(iota_np), "id16": ident,
        })
    res = run_bass_kernel_spmd(nc, in_maps, core_ids=list(range(NC)),
                               trace=_trace, trace_cores=list(range(NC)))
    LAST_EXEC_NS = res.exec_time_ns
    if res.instructions_and_trace is not None:
        LAST_TRACE_PATH = res.instructions_and_trace[1]
    out = np.concatenate([res.results[c]["out"][:shard] for c in range(NC)], 0)
    return out.astype(np.float32)


if __name__ == "__main__":
    import reference
    inp = {k_: np.asarray(v) for k_, v in reference.setup_inputs().items()}
    got = kernel(**inp)
    exp = np.asarray(reference.reference(**inp))
    print(f"Relative error: {np.linalg.norm(got - exp) / np.linalg.norm(exp):.4e}")
